# revision 15
# baseline (speedup 1.0000x reference)
"""Newton-Jacobi batched solver for Trainium2, 8 NeuronCores, data parallel.

Math: the reference's Newton-SOR while-loop always runs MAXITER=16
iterations and converges to the fixed point F(x*)=0; omega only shapes
the reference's PATH, not x*. So the kernel uses undamped Newton-Jacobi
(omega=1), which contracts at ~0.14-0.28 per matvec instead of the
reference's ~0.57 at omega~0.5, plus a device-side warm start
x_init = b * (1/diag(A)) that replaces ~2 iterations. K=2 total matvec
applies reach rel err ~3.6e-3 (vs the 2e-2 gate); r = 1/(d+3*x_init^2)
uses the DVE table reciprocal (a Newton-refined 1/d is NOT enough: the
3x^2/d tail makes the approximation error quadratic-in-0.4, which costs
~3e-3 on the final half-step).

Per apply (per tile, all state transposed [var, element]):
    v~ = (F * r) / 32  rounded to bf16; the SAME rounded vector feeds
         both the x update and the F update, keeping F exactly
         consistent with x (rounding only perturbs the path).
    x' = x - 32 v~
    F' = F - W@v~ - d . (32 v~) + (x'^3 - x^3)
where W = e3m4(32 * offdiag(A)) rides as fp8 stationary weights (their
quantization shifts the fixed point ~1.8e-3 rel; the exact fp32
diagonal d is applied via GpSimd), and the final half-step is
x_out = x' - r*F' in fp32 (nothing consumes F afterwards).

The heavy op is 256 independent 128x128 matvecs per apply per core:
LDWEIGHTS(fp8,FWL)+MATMUL(N=1) pairs at ~24-28ns/element. Hardware
lessons baked in: PSUM accumulation groups serialize per 2KB
zero-region, so each ps tile is padded to a full PSUM bank; ALL DMA
must ride the gpsimd SWDGE queue (~280-315GB/s; the HWDGE rings do
~18-31GB/s on these patterns and their trickle slows other engines);
SWDGE completion semaphores lag the data by 2-4us (growing with queue
depth), so tiles are sized UNEVENLY (80/80/56/40) so that little work
trails the last weight chunk; DVE SBUF traffic steals PE weight-stream
bandwidth, so pointwise work is spread across GpSimd/Scalar. PE unit
order i0 i1 j0 j1 i2 j2 i3 j3 hides drain latency and gives late
tiles' weights maximum DMA slack.
"""

import numpy as np
import ml_dtypes

BATCH = 2048
N = 128
NCORES = 8
PER_CORE = BATCH // NCORES          # 256
TS = (80, 80, 56, 40)               # uneven tile sizes
TO = (0, 80, 160, 216)              # tile offsets
NTILES = 4
WSCALE = 32.0                       # e3m4 weight scale (power of 2: exact)
PSB = 512                           # ps tile padded to 2KB zero region

_BF16 = ml_dtypes.bfloat16
_E3M4 = ml_dtypes.float8_e3m4

_compiled = None


def _build():
    import concourse.bacc as bacc
    import concourse.mybir as mybir
    from concourse.tile import TileContext

    f32 = mybir.dt.float32
    bf16 = mybir.dt.bfloat16
    e3m4 = mybir.dt.float8e3
    op = mybir.AluOpType

    nc = bacc.Bacc("TRN2", target_bir_lowering=False, debug=False)

    wt_d = [
        nc.dram_tensor(f"wt_{t}", [N, TS[t] * N], e3m4, kind="ExternalInput")
        for t in range(NTILES)
    ]
    b_d = nc.dram_tensor("bt", [N, PER_CORE], f32, kind="ExternalInput")
    da_d = nc.dram_tensor("dat", [N, PER_CORE], f32, kind="ExternalInput")
    rd_d = nc.dram_tensor("rdt", [N, PER_CORE], f32, kind="ExternalInput")
    out_d = nc.dram_tensor("outt", [N, PER_CORE], f32, kind="ExternalOutput")

    with TileContext(nc) as tc:
        with (
            tc.tile_pool(name="wts", bufs=1) as wts,
            tc.tile_pool(name="vec", bufs=1) as vec,
            tc.tile_pool(name="roll", bufs=2) as roll,
            tc.tile_pool(name="ps", bufs=2, space="PSUM") as psp,
        ):
            # everything on the gpsimd SWDGE queue, in dependency order:
            # b+rd gate the init chain, tile0's leading chunk gates the
            # first MM, da is needed a bit later, then ~0.5MB chunks.
            b_sb = vec.tile([N, PER_CORE], f32, name="bsb")
            nc.gpsimd.dma_start(b_sb[:, :], b_d[:, :])
            rd_sb = vec.tile([N, PER_CORE], f32, name="rdsb")
            nc.gpsimd.dma_start(rd_sb[:, :], rd_d[:, :])

            w_sb = [
                wts.tile([N, TS[t] * N], e3m4, name=f"wsb{t}", tag=f"w{t}")
                for t in range(NTILES)
            ]
            q0 = 20 * N
            nc.gpsimd.dma_start(w_sb[0][:, :q0], wt_d[0][:, :q0])
            nc.gpsimd.dma_start(w_sb[0][:, q0 : 2 * q0], wt_d[0][:, q0 : 2 * q0])
            da_sb = vec.tile([N, PER_CORE], f32, name="dasb")
            nc.gpsimd.dma_start(da_sb[:, :], da_d[:, :])
            nc.gpsimd.dma_start(w_sb[0][:, 2 * q0 :], wt_d[0][:, 2 * q0 :])
            for t in range(1, NTILES):
                half = (TS[t] // 2) * N
                nc.gpsimd.dma_start(w_sb[t][:, :half], wt_d[t][:, :half])
                nc.gpsimd.dma_start(w_sb[t][:, half:], wt_d[t][:, half:])

            # ---- init chain (full 256-wide; only xi->xb gates the PE) ----
            xi = vec.tile([N, PER_CORE], f32, name="xi")
            nc.vector.tensor_mul(xi[:, :], b_sb[:, :], rd_sb[:, :])
            xb = vec.tile([N, PER_CORE], bf16, name="xb")
            nc.vector.tensor_scalar_mul(xb[:, :], xi[:, :], 1.0 / WSCALE)
            x = vec.tile([N, PER_CORE], f32, name="x")
            nc.scalar.mul(x[:, :], xb[:, :], WSCALE)
            x2 = vec.tile([N, PER_CORE], f32, name="x2")
            nc.scalar.square(x2[:, :], x[:, :])
            da32 = vec.tile([N, PER_CORE], f32, name="da32")
            nc.scalar.mul(da32[:, :], da_sb[:, :], WSCALE)
            x3 = vec.tile([N, PER_CORE], f32, name="x3")
            nc.vector.tensor_mul(x3[:, :], x2[:, :], x[:, :])
            # F = x^3 - b + d.x  (the W@x part lands from PSUM per tile)
            F = vec.tile([N, PER_CORE], f32, name="F")
            nc.vector.tensor_sub(F[:, :], x3[:, :], b_sb[:, :])
            tdx = vec.tile([N, PER_CORE], f32, name="tdx")
            nc.vector.tensor_mul(tdx[:, :], x[:, :], da_sb[:, :])
            nc.vector.tensor_add(F[:, :], F[:, :], tdx[:, :])
            # r = 1/(d + 3 x^2): exact table reciprocal, sliced per tile so
            # r[tile0] is ready before tile0's first drain
            dt_ = vec.tile([N, PER_CORE], f32, name="dt")
            nc.vector.scalar_tensor_tensor(
                dt_[:, :], x2[:, :], 3.0, da_sb[:, :], op0=op.mult, op1=op.add
            )
            r = vec.tile([N, PER_CORE], f32, name="r")
            for t in range(NTILES):
                cs = slice(TO[t], TO[t] + TS[t])
                nc.vector.reciprocal(r[:, cs], dt_[:, cs])

            out_sb = vec.tile([N, PER_CORE], f32, name="outsb")

            vcur = [None] * NTILES

            def apply_mms(ps, a_sb, v_bf, off, npe):
                for e in range(npe):
                    nc.tensor.matmul(
                        ps[:, e : e + 1],
                        a_sb[:, e * N : (e + 1) * N],
                        v_bf[:, off + e : off + e + 1],
                        start=True,
                        stop=True,
                    )

            def emit_init_apply(t):
                cs = slice(TO[t], TO[t] + TS[t])
                ps = psp.tile([N, PSB], f32, name=f"psi{t}", tag=f"ps{t}")
                apply_mms(ps, w_sb[t], xb, TO[t], TS[t])
                v_bf = roll.tile([N, TS[t]], bf16, name=f"vbi{t}", tag=f"vb{t}")
                nc.vector.tensor_add(F[:, cs], F[:, cs], ps[:, : TS[t]])
                nc.vector.scalar_tensor_tensor(
                    v_bf[:, :], F[:, cs], 1.0 / WSCALE, r[:, cs],
                    op0=op.mult, op1=op.mult,
                )
                vcur[t] = v_bf

            def emit_final_apply(t, nsplit=1):
                cs = slice(TO[t], TO[t] + TS[t])
                v_bf = vcur[t]
                ps = psp.tile([N, PSB], f32, name=f"psf{t}", tag=f"ps{t}")
                apply_mms(ps, w_sb[t], v_bf, 0, TS[t])
                # hoisted pointwise (hides under the PE stream):
                # xn = x - 32 v~;  dc2 = (xn^3 - x^3) - d.(32 v~)
                xn = roll.tile([N, TS[t]], f32, name=f"xn{t}", tag=f"xn{t}")
                nc.vector.scalar_tensor_tensor(
                    xn[:, :], v_bf[:, :], -WSCALE, x[:, cs], op0=op.mult, op1=op.add
                )
                x2n = roll.tile([N, TS[t]], f32, name=f"x2n{t}", tag=f"x2n{t}")
                nc.scalar.square(x2n[:, :], xn[:, :])
                x3n = roll.tile([N, TS[t]], f32, name=f"x3n{t}", tag=f"x3n{t}")
                nc.gpsimd.tensor_mul(x3n[:, :], x2n[:, :], xn[:, :])
                dc = roll.tile([N, TS[t]], f32, name=f"dc{t}", tag=f"dc{t}")
                nc.gpsimd.tensor_sub(dc[:, :], x3n[:, :], x3[:, cs])
                tdv = roll.tile([N, TS[t]], f32, name=f"tdv{t}", tag=f"tdv{t}")
                nc.gpsimd.tensor_mul(tdv[:, :], v_bf[:, :], da32[:, cs])
                dc2 = roll.tile([N, TS[t]], f32, name=f"dc2{t}", tag=f"dc2{t}")
                nc.gpsimd.tensor_sub(dc2[:, :], dc[:, :], tdv[:, :])
                # F' = F + dc2 - ps, then the final half-step x_out = xn - r*F'
                nc.vector.tensor_add(F[:, cs], F[:, cs], dc2[:, :])
                rf = roll.tile([N, TS[t]], f32, name=f"rf{t}", tag=f"rf{t}")
                hw = TS[t] // nsplit
                for h in range(nsplit):
                    hs = slice(h * hw, (h + 1) * hw)
                    gs = slice(TO[t] + h * hw, TO[t] + (h + 1) * hw)
                    nc.vector.tensor_sub(F[:, gs], F[:, gs], ps[:, hs])
                    nc.vector.tensor_mul(rf[:, hs], F[:, gs], r[:, gs])
                    nc.vector.scalar_tensor_tensor(
                        out_sb[:, gs], rf[:, hs], -1.0, xn[:, hs],
                        op0=op.mult, op1=op.add,
                    )

            emit_init_apply(0)
            emit_init_apply(1)
            emit_final_apply(0)
            emit_final_apply(1)
            emit_init_apply(2)
            emit_final_apply(2)
            # tiles 0-2 finished: ship their output while tile3 runs
            nc.gpsimd.dma_start(out_d[:, : TO[3]], out_sb[:, : TO[3]])
            emit_init_apply(3)
            emit_final_apply(3, nsplit=2)
            nc.gpsimd.dma_start(out_d[:, TO[3] :], out_sb[:, TO[3] :])

    nc.compile()
    return nc


def _get_compiled():
    global _compiled
    if _compiled is None:
        _compiled = _build()
    return _compiled


def _prep_inputs(x, A, b, omega):
    """Host-side shard + layout/dtype prep. Returns list of per-core maps."""
    A = np.ascontiguousarray(A, dtype=np.float32)
    b = np.asarray(b, dtype=np.float32)
    d = np.ascontiguousarray(np.einsum("bii->bi", A))
    W = A * WSCALE
    idx = np.arange(N)
    W[:, idx, idx] = 0.0
    np.clip(W, -15.5, 15.5, out=W)
    W = W.astype(_E3M4)

    in_maps = []
    for c in range(NCORES):
        sl = slice(c * PER_CORE, (c + 1) * PER_CORE)
        m = {}
        for t in range(NTILES):
            ts = slice(c * PER_CORE + TO[t], c * PER_CORE + TO[t] + TS[t])
            # lhsT layout [j, (e, i)]: element e's weights = W[e].T
            m[f"wt_{t}"] = np.ascontiguousarray(
                W[ts].transpose(2, 0, 1)
            ).reshape(N, TS[t] * N)
        m["bt"] = np.ascontiguousarray(b[sl].T)
        m["dat"] = np.ascontiguousarray(d[sl].T)
        m["rdt"] = np.ascontiguousarray((1.0 / d[sl]).T)
        in_maps.append(m)
    return in_maps


def _run(inputs, trace=False):
    from concourse.bass_utils import run_bass_kernel_spmd

    nc = _get_compiled()
    in_maps = _prep_inputs(inputs["x"], inputs["A"], inputs["b"], inputs["omega"])
    res = run_bass_kernel_spmd(
        nc, in_maps, core_ids=list(range(NCORES)), trace=trace
    )
    out = np.empty((BATCH, N), dtype=np.float32)
    for c in range(NCORES):
        out[c * PER_CORE : (c + 1) * PER_CORE] = res.results[c]["outt"].T
    return out, res


def kernel(x, A, b, omega):
    out, _ = _run({"x": x, "A": A, "b": b, "omega": omega}, trace=False)
    return out


# revision 16
# speedup vs baseline: 1.0821x; 1.0821x over previous
"""Newton-Jacobi batched solver for Trainium2, 8 NeuronCores, data parallel.

Math: the reference's Newton-SOR while-loop always runs MAXITER=16
iterations and converges to the fixed point F(x*)=0; omega only shapes
the reference's PATH, not x*. So the kernel uses undamped Newton-Jacobi
(omega=1), which contracts at ~0.14-0.28 per matvec instead of the
reference's ~0.57 at omega~0.5, plus a device-side warm start
x_init = b * (1/diag(A)) that replaces ~2 iterations. K=2 total matvec
applies reach rel err ~3.6e-3 (vs the 2e-2 gate); r = 1/(d+3*x_init^2)
uses the DVE table reciprocal (a Newton-refined 1/d is NOT enough: the
3x^2/d tail makes the approximation error quadratic-in-0.4, which costs
~3e-3 on the final half-step).

Per apply (per tile, all state transposed [var, element]):
    v~ = (F * r) / 32  rounded to bf16; the SAME rounded vector feeds
         both the x update and the F update, keeping F exactly
         consistent with x (rounding only perturbs the path).
    x' = x - 32 v~
    F' = F - W@v~ - d . (32 v~) + (x'^3 - x^3)
where W = e3m4(32 * offdiag(A)) rides as fp8 stationary weights (their
quantization shifts the fixed point ~1.8e-3 rel; the exact fp32
diagonal d is applied via GpSimd), and the final half-step is
x_out = x' - r*F' in fp32 (nothing consumes F afterwards).

The heavy op is 256 independent 128x128 matvecs per apply per core:
LDWEIGHTS(fp8,FWL)+MATMUL(N=1) pairs at ~24-28ns/element. Hardware
lessons baked in: PSUM accumulation groups serialize per 2KB
zero-region, so each ps tile is padded to a full PSUM bank; ALL DMA
must ride the gpsimd SWDGE queue (~280-315GB/s; the HWDGE rings do
~18-31GB/s on these patterns and their trickle slows other engines);
SWDGE completion semaphores lag the data by 2-4us (growing with queue
depth), so tiles are sized UNEVENLY (80/80/56/40) so that little work
trails the last weight chunk; DVE SBUF traffic steals PE weight-stream
bandwidth, so pointwise work is spread across GpSimd/Scalar. PE unit
order i0 i1 j0 j1 i2 j2 i3 j3 hides drain latency and gives late
tiles' weights maximum DMA slack.
"""

import numpy as np
import ml_dtypes

BATCH = 2048
N = 128
NCORES = 8
PER_CORE = BATCH // NCORES          # 256
TS = (96, 64, 64, 32)               # uneven tile sizes
TO = (0, 96, 160, 224)              # tile offsets
NTILES = 4
WSCALE = 32.0                       # e3m4 weight scale (power of 2: exact)
PSB = 512                           # ps tile padded to 2KB zero region

_BF16 = ml_dtypes.bfloat16
_E3M4 = ml_dtypes.float8_e3m4

_compiled = None


def _build():
    import concourse.bacc as bacc
    import concourse.mybir as mybir
    from concourse.tile import TileContext

    f32 = mybir.dt.float32
    bf16 = mybir.dt.bfloat16
    e3m4 = mybir.dt.float8e3
    op = mybir.AluOpType

    nc = bacc.Bacc("TRN2", target_bir_lowering=False, debug=False)

    wt_d = [
        nc.dram_tensor(f"wt_{t}", [N, TS[t] * N], e3m4, kind="ExternalInput")
        for t in range(NTILES)
    ]
    b_d = nc.dram_tensor("bt", [N, PER_CORE], f32, kind="ExternalInput")
    da_d = nc.dram_tensor("dat", [N, PER_CORE], f32, kind="ExternalInput")
    rd_d = nc.dram_tensor("rdt", [N, PER_CORE], f32, kind="ExternalInput")
    out_d = nc.dram_tensor("outt", [N, PER_CORE], f32, kind="ExternalOutput")

    with TileContext(nc) as tc:
        with (
            tc.tile_pool(name="wts", bufs=1) as wts,
            tc.tile_pool(name="vec", bufs=1) as vec,
            tc.tile_pool(name="roll", bufs=2) as roll,
            tc.tile_pool(name="ps", bufs=2, space="PSUM") as psp,
        ):
            # everything on the gpsimd SWDGE queue, in dependency order:
            # b+rd gate the init chain, tile0's leading chunk gates the
            # first MM, da is needed a bit later, then ~0.5MB chunks.
            b_sb = vec.tile([N, PER_CORE], f32, name="bsb")
            nc.gpsimd.dma_start(b_sb[:, :], b_d[:, :])
            rd_sb = vec.tile([N, PER_CORE], f32, name="rdsb")
            nc.gpsimd.dma_start(rd_sb[:, :], rd_d[:, :])

            w_sb = [
                wts.tile([N, TS[t] * N], e3m4, name=f"wsb{t}", tag=f"w{t}")
                for t in range(NTILES)
            ]
            # uniform 512KB chunks (32 elements, 4KB rows)
            CK = 32 * N
            nc.gpsimd.dma_start(w_sb[0][:, :CK], wt_d[0][:, :CK])
            nc.gpsimd.dma_start(w_sb[0][:, CK : 2 * CK], wt_d[0][:, CK : 2 * CK])
            da_sb = vec.tile([N, PER_CORE], f32, name="dasb")
            nc.gpsimd.dma_start(da_sb[:, :], da_d[:, :])
            nc.gpsimd.dma_start(w_sb[0][:, 2 * CK :], wt_d[0][:, 2 * CK :])
            for t in range(1, NTILES):
                for q in range(0, TS[t] * N, CK):
                    nc.gpsimd.dma_start(
                        w_sb[t][:, q : q + CK], wt_d[t][:, q : q + CK]
                    )

            # ---- init chain (full 256-wide; only xi->xb gates the PE) ----
            xi = vec.tile([N, PER_CORE], f32, name="xi")
            nc.vector.tensor_mul(xi[:, :], b_sb[:, :], rd_sb[:, :])
            xb = vec.tile([N, PER_CORE], bf16, name="xb")
            nc.vector.tensor_scalar_mul(xb[:, :], xi[:, :], 1.0 / WSCALE)
            x = vec.tile([N, PER_CORE], f32, name="x")
            nc.scalar.mul(x[:, :], xb[:, :], WSCALE)
            x2 = vec.tile([N, PER_CORE], f32, name="x2")
            nc.scalar.square(x2[:, :], x[:, :])
            da32 = vec.tile([N, PER_CORE], f32, name="da32")
            nc.scalar.mul(da32[:, :], da_sb[:, :], WSCALE)
            x3 = vec.tile([N, PER_CORE], f32, name="x3")
            nc.vector.tensor_mul(x3[:, :], x2[:, :], x[:, :])
            # F = x^3 - b + d.x  (the W@x part lands from PSUM per tile)
            F = vec.tile([N, PER_CORE], f32, name="F")
            nc.vector.tensor_sub(F[:, :], x3[:, :], b_sb[:, :])
            tdx = vec.tile([N, PER_CORE], f32, name="tdx")
            nc.vector.tensor_mul(tdx[:, :], x[:, :], da_sb[:, :])
            nc.vector.tensor_add(F[:, :], F[:, :], tdx[:, :])
            # r = 1/(d + 3 x^2): exact table reciprocal, sliced per tile so
            # r[tile0] is ready before tile0's first drain
            dt_ = vec.tile([N, PER_CORE], f32, name="dt")
            nc.vector.scalar_tensor_tensor(
                dt_[:, :], x2[:, :], 3.0, da_sb[:, :], op0=op.mult, op1=op.add
            )
            r = vec.tile([N, PER_CORE], f32, name="r")
            for t in range(NTILES):
                cs = slice(TO[t], TO[t] + TS[t])
                nc.vector.reciprocal(r[:, cs], dt_[:, cs])

            out_sb = vec.tile([N, PER_CORE], f32, name="outsb")

            vcur = [None] * NTILES

            def apply_mms(ps, a_sb, v_bf, off, npe):
                for e in range(npe):
                    nc.tensor.matmul(
                        ps[:, e : e + 1],
                        a_sb[:, e * N : (e + 1) * N],
                        v_bf[:, off + e : off + e + 1],
                        start=True,
                        stop=True,
                    )

            def emit_init_apply(t):
                cs = slice(TO[t], TO[t] + TS[t])
                ps = psp.tile([N, PSB], f32, name=f"psi{t}", tag=f"ps{t}")
                apply_mms(ps, w_sb[t], xb, TO[t], TS[t])
                v_bf = roll.tile([N, TS[t]], bf16, name=f"vbi{t}", tag=f"vb{t}")
                nc.vector.tensor_add(F[:, cs], F[:, cs], ps[:, : TS[t]])
                nc.vector.scalar_tensor_tensor(
                    v_bf[:, :], F[:, cs], 1.0 / WSCALE, r[:, cs],
                    op0=op.mult, op1=op.mult,
                )
                vcur[t] = v_bf

            def emit_final_apply(t, nsplit=1):
                cs = slice(TO[t], TO[t] + TS[t])
                v_bf = vcur[t]
                ps = psp.tile([N, PSB], f32, name=f"psf{t}", tag=f"ps{t}")
                apply_mms(ps, w_sb[t], v_bf, 0, TS[t])
                # hoisted pointwise (hides under the PE stream):
                # xn = x - 32 v~;  dc2 = (xn^3 - x^3) - d.(32 v~)
                xn = roll.tile([N, TS[t]], f32, name=f"xn{t}", tag=f"xn{t}")
                nc.vector.scalar_tensor_tensor(
                    xn[:, :], v_bf[:, :], -WSCALE, x[:, cs], op0=op.mult, op1=op.add
                )
                x2n = roll.tile([N, TS[t]], f32, name=f"x2n{t}", tag=f"x2n{t}")
                nc.scalar.square(x2n[:, :], xn[:, :])
                x3n = roll.tile([N, TS[t]], f32, name=f"x3n{t}", tag=f"x3n{t}")
                nc.gpsimd.tensor_mul(x3n[:, :], x2n[:, :], xn[:, :])
                dc = roll.tile([N, TS[t]], f32, name=f"dc{t}", tag=f"dc{t}")
                nc.gpsimd.tensor_sub(dc[:, :], x3n[:, :], x3[:, cs])
                tdv = roll.tile([N, TS[t]], f32, name=f"tdv{t}", tag=f"tdv{t}")
                nc.gpsimd.tensor_mul(tdv[:, :], v_bf[:, :], da32[:, cs])
                dc2 = roll.tile([N, TS[t]], f32, name=f"dc2{t}", tag=f"dc2{t}")
                nc.gpsimd.tensor_sub(dc2[:, :], dc[:, :], tdv[:, :])
                # F' = F + dc2 - ps, then the final half-step x_out = xn - r*F'
                nc.vector.tensor_add(F[:, cs], F[:, cs], dc2[:, :])
                rf = roll.tile([N, TS[t]], f32, name=f"rf{t}", tag=f"rf{t}")
                hw = TS[t] // nsplit
                for h in range(nsplit):
                    hs = slice(h * hw, (h + 1) * hw)
                    gs = slice(TO[t] + h * hw, TO[t] + (h + 1) * hw)
                    nc.vector.tensor_sub(F[:, gs], F[:, gs], ps[:, hs])
                    nc.vector.tensor_mul(rf[:, hs], F[:, gs], r[:, gs])
                    nc.vector.scalar_tensor_tensor(
                        out_sb[:, gs], rf[:, hs], -1.0, xn[:, hs],
                        op0=op.mult, op1=op.add,
                    )

            emit_init_apply(0)
            emit_init_apply(1)
            emit_final_apply(0)
            emit_final_apply(1)
            emit_init_apply(2)
            emit_final_apply(2)
            # tiles 0-2 finished: ship their output while tile3 runs
            nc.gpsimd.dma_start(out_d[:, : TO[3]], out_sb[:, : TO[3]])
            emit_init_apply(3)
            emit_final_apply(3)
            # tiny final slice on the (otherwise idle) sync queue: lower
            # trigger->data latency than the SWDGE path at this point
            nc.sync.dma_start(out_d[:, TO[3] :], out_sb[:, TO[3] :])

    nc.compile()
    return nc


def _get_compiled():
    global _compiled
    if _compiled is None:
        _compiled = _build()
    return _compiled


def _prep_inputs(x, A, b, omega):
    """Host-side shard + layout/dtype prep. Returns list of per-core maps."""
    A = np.ascontiguousarray(A, dtype=np.float32)
    b = np.asarray(b, dtype=np.float32)
    d = np.ascontiguousarray(np.einsum("bii->bi", A))
    W = A * WSCALE
    idx = np.arange(N)
    W[:, idx, idx] = 0.0
    np.clip(W, -15.5, 15.5, out=W)
    W = W.astype(_E3M4)

    in_maps = []
    for c in range(NCORES):
        sl = slice(c * PER_CORE, (c + 1) * PER_CORE)
        m = {}
        for t in range(NTILES):
            ts = slice(c * PER_CORE + TO[t], c * PER_CORE + TO[t] + TS[t])
            # lhsT layout [j, (e, i)]: element e's weights = W[e].T
            m[f"wt_{t}"] = np.ascontiguousarray(
                W[ts].transpose(2, 0, 1)
            ).reshape(N, TS[t] * N)
        m["bt"] = np.ascontiguousarray(b[sl].T)
        m["dat"] = np.ascontiguousarray(d[sl].T)
        m["rdt"] = np.ascontiguousarray((1.0 / d[sl]).T)
        in_maps.append(m)
    return in_maps


def _run(inputs, trace=False):
    from concourse.bass_utils import run_bass_kernel_spmd

    nc = _get_compiled()
    in_maps = _prep_inputs(inputs["x"], inputs["A"], inputs["b"], inputs["omega"])
    res = run_bass_kernel_spmd(
        nc, in_maps, core_ids=list(range(NCORES)), trace=trace
    )
    out = np.empty((BATCH, N), dtype=np.float32)
    for c in range(NCORES):
        out[c * PER_CORE : (c + 1) * PER_CORE] = res.results[c]["outt"].T
    return out, res


def kernel(x, A, b, omega):
    out, _ = _run({"x": x, "A": A, "b": b, "omega": omega}, trace=False)
    return out
